# revision 1
# baseline (speedup 1.0000x reference)
"""Multi-head attention (B=2, S=4096, HIDDEN=512, HEADS=8) on 8 TRN2 NeuronCores.

Sharding: 8 cores = 2 batches x 4 head-groups (2 heads each).
Core c handles batch b = c//4 and heads {2g, 2g+1} where g = c%4
(projection feature slice [g*128, (g+1)*128)).

Per-core kernel (single SPMD program, different input data per core):
  - Q^T/K^T/V^T projections from pre-transposed x^T (host supplies x[b].T),
    V^T then PE-transposed into natural [t, d] layout
  - scores computed transposed: S^T[t, s] = sum_d K^T[d,t] Q^T[d,s]
  - P^T = exp(S^T / 8) on ScalarE straight out of PSUM, two banks per op
  - PV matmul with a ones-column appended to V so the softmax denominator
    l[s] drops out of the same accumulation (row 64 of the ctx PSUM tile)
  - normalization: r = 1/l broadcast across partitions via a K=1 matmul
  - output projection vs Wo^T rows of this core's heads -> partial [S, 512]
Host sums the 4 partials per batch and adds bo.

Large matmuls run in float32r (fast fp32 mode, full PE rate at N>=512).
float32r is a real packed format: every fp32r operand is produced by a
compute instruction writing a float32r tile (DMA'd fp32 data is converted
once on VectorE).
"""

import sys

import numpy as np

B, S, HID, HEADS, HD = 2, 4096, 512, 8, 64
FSL = 128          # features per core = 2 heads * 64
NCORES = 8
QC = 512           # query-chunk width
NTB = S // 128     # 32 key blocks
NQC = S // QC      # 8 query chunks

_PROGRAM = None


def _ensure_imports():
    try:
        import concourse  # noqa: F401
    except ImportError:
        sys.path.insert(0, "/opt/trn_rl_repo")


def _build_program():
    _ensure_imports()
    import concourse.bacc as bacc
    import concourse.mybir as mybir
    import concourse.tile as tile
    from concourse.masks import make_identity

    f32 = mybir.dt.float32
    f32r = mybir.dt.float32r
    AF = mybir.ActivationFunctionType

    nc = bacc.Bacc(
        "TRN2",
        target_bir_lowering=False,
        debug=False,
        enable_asserts=False,
        num_devices=NCORES,
    )

    xT = nc.dram_tensor("xT", [HID, S], f32, kind="ExternalInput").ap()
    wqT = nc.dram_tensor("wqT", [HID, FSL], f32, kind="ExternalInput").ap()
    wkT = nc.dram_tensor("wkT", [HID, FSL], f32, kind="ExternalInput").ap()
    wvT = nc.dram_tensor("wvT", [HID, FSL], f32, kind="ExternalInput").ap()
    woT = nc.dram_tensor("woT", [FSL, HID], f32, kind="ExternalInput").ap()
    bq = nc.dram_tensor("bq", [FSL, 1], f32, kind="ExternalInput").ap()
    bk = nc.dram_tensor("bk", [FSL, 1], f32, kind="ExternalInput").ap()
    bv = nc.dram_tensor("bv", [FSL, 1], f32, kind="ExternalInput").ap()
    out = nc.dram_tensor("out", [S, HID], f32, kind="ExternalOutput").ap()

    with tile.TileContext(nc) as tc:
        with (
            tc.tile_pool(name="persist", bufs=1) as pp,
            tc.tile_pool(name="vp_pool", bufs=NTB) as vpp,
        ):
            kt = pp.tile([FSL, S], f32r, tag="kt")
            qt = pp.tile([FSL, S], f32r, tag="qt")
            ctx01 = pp.tile([FSL, S], f32r, tag="ctx01")
            wo_r = pp.tile([FSL, HID], f32r, tag="wo_r")
            ones = pp.tile([128, 128], f32, tag="ones")
            ident = pp.tile([128, 128], f32r, tag="ident")
            bq_sb = pp.tile([FSL, 1], f32, tag="bq_sb")
            bk_sb = pp.tile([FSL, 1], f32, tag="bk_sb")
            bv_sb = pp.tile([FSL, 1], f32, tag="bv_sb")

            # biases ride the SWDGE ring so they don't occupy the HWDGE
            # rings' heads (each HWDGE descriptor costs ~0.6us regardless
            # of size)
            nc.gpsimd.dma_start(bq_sb[:], bq[:])
            nc.gpsimd.dma_start(bk_sb[:], bk[:])
            nc.gpsimd.dma_start(bv_sb[:], bv[:])
            nc.vector.memset(ones[:], 1.0)

            # warm the ACT exp table during the initial DMA window
            warm = pp.tile([128, 1], f32, tag="warm")
            nc.scalar.activation(warm[:], ones[:, 0:1], AF.Exp)

            vp_tiles = []

            with (
                tc.tile_pool(name="w_pool", bufs=1) as wp,
                tc.tile_pool(name="psB", bufs=1, space="PSUM") as psB,
                tc.tile_pool(name="pt_pool", bufs=9) as ptp,
                tc.tile_pool(name="small", bufs=3) as sp,
                tc.tile_pool(name="out_pool", bufs=4) as obp,
            ):
                # ---------------- projections ----------------
                ident_f32 = wp.tile([128, 128], f32, tag="ident_f32")
                make_identity(nc, ident_f32[:])
                nc.vector.tensor_copy(ident[:], ident_f32[:])

                # weights first: tiny DMAs must not queue behind the x chunks
                # (k/q before v; wo last -- it is not needed until the first
                # output projection)
                wqs, wks, wvs = [], [], []
                for name, dst, src in (("wk", wks, wkT), ("wq", wqs, wqT),
                                       ("wv", wvs, wvT)):
                    # one DMA + one convert per weight matrix: [512,128] DRAM
                    # folded to [128, 4, 128] (partition p, chunk i)
                    w_raw = wp.tile([128, 4, FSL], f32, tag="wraw", bufs=2)
                    nc.sync.dma_start(w_raw[:],
                                      src.rearrange("(i p) f -> p i f", p=128))
                    w_r = wp.tile([128, 4, FSL], f32r, tag=f"{name}r")
                    nc.scalar.copy(w_r[:], w_raw[:])
                    for i in range(4):
                        dst.append(w_r[:, i, :])

                # per-t8 pipeline: load x chunks (both HWDGE rings), convert
                # to rotating fp32r chunks, project K^T/Q^T/V^T, transpose V
                def emit_attn_pair(qc, h, tb2, ctx_pss):
                    hh = h * HD
                    st = psB.tile([128, 2, QC], f32, tag="st", bufs=2,
                                  name="st")
                    for j in range(2):
                        tb = tb2 * 2 + j
                        nc.tensor.matmul(
                            st[:, j, :],
                            kt[hh:hh + HD, tb * 128:(tb + 1) * 128],
                            qt[hh:hh + HD, qc * QC:(qc + 1) * QC],
                            start=True, stop=True)
                    pt = ptp.tile([128, 2, QC], f32r, tag="pt", name="pt")
                    nc.scalar.activation(pt[:], st[:], AF.Exp,
                                         scale=float(HD) ** -0.5)
                    for j in range(2):
                        tb = tb2 * 2 + j
                        nc.tensor.matmul(
                            ctx_pss[h][:],
                            vp_tiles[tb][:, h * (HD + 1):(h + 1) * (HD + 1)],
                            pt[:, j, :],
                            start=(tb == 0), stop=(tb == NTB - 1))

                def emit_attn_tail(qc, ctx_pss):
                    for h in range(2):
                        ctx_ps = ctx_pss[h]
                        # normalize: r = 1/l lives on partition HD (=64);
                        # reciprocal moves it to partition 0 (1-partition op,
                        # quadrant-aligned src -- the HW-verified case), then
                        # GPSIMD broadcasts it across the 64 ctx partitions
                        r_t = sp.tile([128, QC], f32, tag="r", name="r_t")
                        nc.vector.reciprocal(r_t[0:1, :],
                                             ctx_ps[HD:HD + 1, :])
                        rb_sb = sp.tile([HD, QC], f32, tag="rbs", name="rb_sb")
                        nc.gpsimd.partition_broadcast(rb_sb[:], r_t[0:1, :])
                        qs = slice(qc * QC, (qc + 1) * QC)
                        if h == 0:
                            nc.vector.tensor_mul(
                                ctx01[0:HD, qs], ctx_ps[0:HD, :], rb_sb[:])
                        else:
                            # partition-shift to rows 64-127 via two
                            # HW-verified 32-partition quadrant copies
                            tmp1 = sp.tile([HD, QC], f32r, tag="tmp1",
                                           name="tmp1")
                            nc.vector.tensor_mul(tmp1[:], ctx_ps[0:HD, :],
                                                 rb_sb[:])
                            nc.vector.tensor_copy(ctx01[HD:HD + 32, qs],
                                                  tmp1[0:32, :])
                            nc.vector.tensor_copy(ctx01[HD + 32:FSL, qs],
                                                  tmp1[32:HD, :])
                    for sc in range(QC // 128):
                        col = qc * QC + sc * 128
                        po = psB.tile([128, HID], f32, tag="misc", bufs=2,
                                      name="po")
                        nc.tensor.matmul(po[:], ctx01[:, col:col + 128],
                                         wo_r[:], start=True, stop=True)
                        ob = obp.tile([128, HID], f32, tag="ob", name="ob")
                        if qc == NQC - 1:
                            nc.scalar.copy(ob[:], po[:])
                        else:
                            nc.vector.tensor_copy(ob[:], po[:])
                        nc.sync.dma_start(out[col:col + 128, :], ob[:])

                ctx_q0 = [
                    psB.tile([HD + 1, QC], f32, tag="ctx", bufs=2,
                             name=f"ctxq0{h}")
                    for h in range(2)
                ]
                ctx_q1 = [
                    psB.tile([HD + 1, QC], f32, tag="ctx", bufs=2,
                             name=f"ctxq1{h}")
                    for h in range(2)
                ]
                Q1_EARLY = 4
                for t8 in range(NQC):
                    cs = slice(t8 * QC, (t8 + 1) * QC)
                    xcs = []
                    for i in range(4):
                        xt_raw = wp.tile([128, QC], f32, tag="xtraw", bufs=10)
                        eng = nc.sync if i % 2 == 0 else nc.scalar
                        eng.dma_start(xt_raw[:], xT[i * 128:(i + 1) * 128, cs])
                        xc = wp.tile([128, QC], f32r, tag="xc", bufs=8)
                        if t8 < 3:
                            nc.scalar.copy(xc[:], xt_raw[:])
                        else:
                            nc.vector.tensor_copy(xc[:], xt_raw[:])
                        xcs.append(xc)
                    for ws, bias_sb, dst in ((wks, bk_sb, kt), (wqs, bq_sb, qt)):
                        ps = psB.tile([FSL, QC], f32, tag="misc", bufs=2,
                                      name="kqps")
                        for i in range(4):
                            nc.tensor.matmul(
                                ps[:], ws[i][:], xcs[i][:],
                                start=(i == 0), stop=(i == 3))
                        nc.vector.tensor_scalar_add(dst[:, cs], ps[:], bias_sb[:])
                    psvT = psB.tile([FSL, QC], f32, tag="misc", bufs=2,
                                    name="vps")
                    for i in range(4):
                        nc.tensor.matmul(
                            psvT[:], wvs[i][:], xcs[i][:],
                            start=(i == 0), stop=(i == 3))
                    vt_sb = wp.tile([FSL, QC], f32r, tag="vt", bufs=2)
                    nc.vector.tensor_scalar_add(vt_sb[:], psvT[:], bv_sb[:])
                    for j in range(QC // 128):
                        vtp = psB.tile([128, FSL], f32r, tag="misc", bufs=2,
                                       name="vtp")
                        nc.tensor.transpose(vtp[:], vt_sb[:, j * 128:(j + 1) * 128],
                                            ident[:])
                        vp = vpp.tile([128, 2 * (HD + 1)], f32r, tag="vp")
                        nc.vector.tensor_copy(vp[:, 0:HD], vtp[:, 0:HD])
                        nc.vector.tensor_copy(vp[:, HD + 1:2 * HD + 1],
                                              vtp[:, HD:2 * HD])
                        nc.vector.tensor_copy(vp[:, HD:HD + 1], ones[:, 0:1])
                        nc.vector.tensor_copy(vp[:, 2 * HD + 1:2 * HD + 2],
                                              ones[:, 0:1])
                        vp_tiles.append(vp)
                    if t8 == 0:
                        # Wo load deferred off the scalar ring's head; it is
                        # not consumed until the first output projection
                        wo_raw = wp.tile([FSL, HID], f32, tag="wo_raw")
                        nc.scalar.dma_start(wo_raw[:], woT[:])
                        nc.vector.tensor_copy(wo_r[:], wo_raw[:])
                    # attention for qc=0 rides along as soon as this t8's
                    # K/Q/V chunks exist, keeping ScalarE fed during the
                    # projection phase
                    for tb2 in (2 * t8, 2 * t8 + 1):
                        for h in range(2):
                            emit_attn_pair(0, h, tb2, ctx_q0)
                    # late projection: qc=1's first score/exp groups ride
                    # along to fill ScalarE's idle windows (their PVs wait
                    # for ctx slots, the exps do not)
                    if t8 >= NQC - Q1_EARLY:
                        tb2e = t8 - (NQC - Q1_EARLY)
                        for h in range(2):
                            emit_attn_pair(1, h, tb2e, ctx_q1)
                emit_attn_tail(0, ctx_q0)

                # ---------------- attention + output projection ----------------
                for qc in range(1, NQC):
                    if qc == 1:
                        ctx_pss = ctx_q1
                        tb2_start = Q1_EARLY
                    else:
                        ctx_pss = [
                            psB.tile([HD + 1, QC], f32, tag="ctx", bufs=2,
                                     name=f"ctxps{h}")
                            for h in range(2)
                        ]
                        tb2_start = 0
                    for tb2 in range(tb2_start, NTB // 2):
                        for h in range(2):
                            emit_attn_pair(qc, h, tb2, ctx_pss)
                    emit_attn_tail(qc, ctx_pss)

    nc.compile()
    return nc


def _get_program():
    global _PROGRAM
    if _PROGRAM is None:
        _PROGRAM = _build_program()
    return _PROGRAM


def kernel(**inputs):
    _ensure_imports()
    from concourse import bass_utils

    x = np.ascontiguousarray(np.asarray(inputs["x"], dtype=np.float32))
    Wq = np.asarray(inputs["Wq"], dtype=np.float32)
    Wk = np.asarray(inputs["Wk"], dtype=np.float32)
    Wv = np.asarray(inputs["Wv"], dtype=np.float32)
    Wo = np.asarray(inputs["Wo"], dtype=np.float32)
    bq = np.asarray(inputs["bq"], dtype=np.float32)
    bk = np.asarray(inputs["bk"], dtype=np.float32)
    bv = np.asarray(inputs["bv"], dtype=np.float32)
    bo = np.asarray(inputs["bo"], dtype=np.float32)

    nc = _get_program()

    wqT_full = np.ascontiguousarray(Wq.T)
    wkT_full = np.ascontiguousarray(Wk.T)
    wvT_full = np.ascontiguousarray(Wv.T)
    woT_full = np.ascontiguousarray(Wo.T)

    in_maps = []
    for c in range(NCORES):
        b, g = divmod(c, 4)
        fs = slice(g * FSL, (g + 1) * FSL)
        in_maps.append({
            "xT": np.ascontiguousarray(x[b].T),
            "wqT": np.ascontiguousarray(wqT_full[:, fs]),
            "wkT": np.ascontiguousarray(wkT_full[:, fs]),
            "wvT": np.ascontiguousarray(wvT_full[:, fs]),
            "woT": np.ascontiguousarray(woT_full[fs, :]),
            "bq": np.ascontiguousarray(bq[fs].reshape(FSL, 1)),
            "bk": np.ascontiguousarray(bk[fs].reshape(FSL, 1)),
            "bv": np.ascontiguousarray(bv[fs].reshape(FSL, 1)),
        })

    res = bass_utils.run_bass_kernel_spmd(nc, in_maps,
                                          core_ids=list(range(NCORES)))
    outs = [r["out"] for r in res.results]

    full = np.empty((B, S, HID), dtype=np.float32)
    for b in range(B):
        full[b] = outs[4 * b] + outs[4 * b + 1] + outs[4 * b + 2] + outs[4 * b + 3]
        full[b] += bo
    return full



# revision 6
# speedup vs baseline: 1.0974x; 1.0974x over previous
"""Multi-head attention (B=2, S=4096, HIDDEN=512, HEADS=8) on 8 TRN2 NeuronCores.

Sharding: 8 cores = 2 batches x 4 head-groups (2 heads each).
Core c handles batch b = c//4 and heads {2g, 2g+1} where g = c%4
(projection feature slice [g*128, (g+1)*128)).

Per-core kernel, all-bf16 matmul datapath (host pre-converts x and the
weights to bf16; PSUM accumulation stays fp32):
  - K^T/Q^T projections from x^T chunks (lhsT = W chunks, N=512)
  - V projected directly into natural [t, d] layout (lhsT = x^T chunk
    slices), with zero-padded W columns + a broadcast bias row so the
    per-head ones column (softmax denominator) appears for free
  - scores S^T[t, s] = K^T-block^T Q^T chunk in PSUM (fp32, exact)
  - softmax exp split across two engines:
      ACT:  exact exp activation (fp32 -> bf16)
      DVE:  one-pass Schraudolph exp2: bf16_bitcast(int16(
        x*(16*log2e) + B)) -- the f32->int16 convert rounds to nearest,
        the int16 bit pattern IS the bf16 exponential approximation
  - PV in swapped orientation: ctx[s, d+1] accumulates pt-block^T @ vp,
    N=65 per matmul (full 128-wide stationary dim) -- half the PE rows
    of the [d, s] orientation.  PSUM accumulation groups are 2KB-bank
    granular in the sim (start=True wipes the whole bank row), so each
    (query-block, head) group runs alone on its bank, all groups of a
    bank at the same columns so the hazard tracker serializes them.
  - normalize: l sits in ctx column 64; per-partition reciprocal +
    tensor_scalar multiply (queries live on partitions here)
  - ctx transposed back to [d, s] by the DMA crossbar (2-byte
    transpose, no PE/DVE cost)
  - output projection per 128-query block -> partial [S, 512]
Host sums the 4 partials per batch and adds bo.
"""

import sys

import numpy as np

B, S, HID, HEADS, HD = 2, 4096, 512, 8, 64
FSL = 128          # features per core = 2 heads * 64
NCORES = 8
QC = 512           # query-chunk width
NTB = S // 128     # 32 key blocks
NQC = S // QC      # 8 query chunks
Q1_EARLY = 4       # qc=1 tb2-pairs whose QK/exp ride the projection phase

LOG2E = 1.4426950408889634
EXP_S = 16.0 * LOG2E          # 128*log2e * scale(1/8)
EXP_B = 16256.0 - 7.5         # zero-mean-tuned Schraudolph constant

# exp engine assignment pattern, indexed by a global tile counter.
# A = ACT exact exp, D = DVE schraudolph (GPSIMD cannot read PSUM).
EXP_PATTERN = ("A", "A", "D", "A", "D", "A", "A", "D", "A", "D", "A", "D", "A")

_PROGRAM = None


def _ensure_imports():
    try:
        import concourse  # noqa: F401
    except ImportError:
        sys.path.insert(0, "/opt/trn_rl_repo")


def _build_program():
    _ensure_imports()
    import concourse.bacc as bacc
    import concourse.mybir as mybir
    import concourse.tile as tile

    f32 = mybir.dt.float32
    bf16 = mybir.dt.bfloat16
    i16 = mybir.dt.int16
    AF = mybir.ActivationFunctionType
    ALU = mybir.AluOpType

    nc = bacc.Bacc(
        "TRN2",
        target_bir_lowering=False,
        debug=False,
        enable_asserts=False,
        num_devices=NCORES,
    )

    xT = nc.dram_tensor("xT", [HID, S], bf16, kind="ExternalInput").ap()
    wqT = nc.dram_tensor("wqT", [HID, FSL], bf16, kind="ExternalInput").ap()
    wkT = nc.dram_tensor("wkT", [HID, FSL], bf16, kind="ExternalInput").ap()
    wvT = nc.dram_tensor("wvT", [HID, 130], bf16, kind="ExternalInput").ap()
    woT = nc.dram_tensor("woT", [FSL, HID], bf16, kind="ExternalInput").ap()
    bq = nc.dram_tensor("bq", [FSL, 1], f32, kind="ExternalInput").ap()
    bk = nc.dram_tensor("bk", [FSL, 1], f32, kind="ExternalInput").ap()
    bvr = nc.dram_tensor("bvr", [1, 130], f32, kind="ExternalInput").ap()
    out = nc.dram_tensor("out", [S, HID], f32, kind="ExternalOutput").ap()

    exp_idx = [0]
    pt_map = {}

    with tile.TileContext(nc) as tc:
        with (
            tc.tile_pool(name="persist", bufs=1) as pp,
            tc.tile_pool(name="vp_pool", bufs=NTB) as vpp,
            tc.tile_pool(name="pspersist", bufs=1, space="PSUM") as psp,
            tc.tile_pool(name="stp", bufs=2, space="PSUM") as stp,
            tc.tile_pool(name="pt_pool", bufs=44) as ptp,
            tc.tile_pool(name="w_pool", bufs=1) as wp,
            tc.tile_pool(name="small", bufs=4) as sp,
            tc.tile_pool(name="out_pool", bufs=4) as obp,
        ):
            kt = pp.tile([FSL, S], bf16, tag="kt")
            qt = pp.tile([FSL, S], bf16, tag="qt")
            wo_sb = pp.tile([FSL, HID], bf16, tag="wo_sb")
            bq_sb = pp.tile([FSL, 1], f32, tag="bq_sb")
            bk_sb = pp.tile([FSL, 1], f32, tag="bk_sb")
            bvr_sb = pp.tile([1, 130], f32, tag="bvr_sb")
            bvb = pp.tile([128, 130], f32, tag="bvb")

            # persistent PSUM banks; one live accumulation group per bank
            # (sim group state is bank-granular):
            #  ctxA/ctxB: PV slot groups, always at cols 0:65
            #  mix: K/Q projection psum (phase A), po output proj (tails)
            #  vpb: V projection psum (phase A)
            ctxA = psp.tile([128, 512], f32, tag="ctxA")
            ctxB = psp.tile([128, 512], f32, tag="ctxB")
            mix = psp.tile([128, 512], f32, tag="mix")
            vpb = psp.tile([128, 512], f32, tag="vpb")
            ctx_banks = (ctxA, ctxB)

            # small DMAs ride the SWDGE ring
            nc.gpsimd.dma_start(bq_sb[:], bq[:])
            nc.gpsimd.dma_start(bk_sb[:], bk[:])
            nc.gpsimd.dma_start(bvr_sb[:], bvr[:])
            nc.gpsimd.partition_broadcast(bvb[:], bvr_sb[0:1, :])

            # weights first on the sync ring: tiny, needed before projections
            wk_t = wp.tile([128, 4, FSL], bf16, tag="wk_t")
            nc.sync.dma_start(wk_t[:], wkT.rearrange("(i p) f -> p i f", p=128))
            wq_t = wp.tile([128, 4, FSL], bf16, tag="wq_t")
            nc.sync.dma_start(wq_t[:], wqT.rearrange("(i p) f -> p i f", p=128))
            wv_t = wp.tile([128, 4, 130], bf16, tag="wv_t")
            nc.sync.dma_start(wv_t[:], wvT.rearrange("(i p) f -> p i f", p=128))
            nc.sync.dma_start(wo_sb[:], woT[:])

            vp_tiles = []

            def emit_qk_exp(qc, h, tb2):
                st = stp.tile([128, 2, QC], f32, tag="st", name="st")
                for j in range(2):
                    tb = tb2 * 2 + j
                    nc.tensor.matmul(
                        st[:, j, :],
                        kt[h * HD:(h + 1) * HD, tb * 128:(tb + 1) * 128],
                        qt[h * HD:(h + 1) * HD, qc * QC:(qc + 1) * QC],
                        start=True, stop=True)
                pt = ptp.tile([128, 2, QC], bf16, tag="pt", name="pt")
                eng = EXP_PATTERN[exp_idx[0] % len(EXP_PATTERN)]
                exp_idx[0] += 1
                if eng == "A":
                    nc.scalar.activation(pt[:], st[:], AF.Exp,
                                         scale=float(HD) ** -0.5)
                else:
                    nc.vector.tensor_scalar(pt[:].bitcast(i16), st[:],
                                            EXP_S, EXP_B, ALU.mult, ALU.add)
                pt_map[(qc, h, tb2)] = pt

            def emit_pv_tail(qc):
                ctxn = [sp.tile([128, 2, HD], bf16, tag=f"cn{sb}",
                                name=f"cn{sb}") for sb in range(4)]
                ctxTs = sp.tile([128, 512], bf16, tag="ctxTs", name="ctxTs")
                for sb in range(4):
                    for h in range(2):
                        bank = ctx_banks[h]
                        slot = bank[:, 0:65]
                        for tb2 in range(NTB // 2):
                            pt = pt_map[(qc, h, tb2)]
                            for j in range(2):
                                tb = tb2 * 2 + j
                                nc.tensor.matmul(
                                    slot,
                                    pt[:, j, sb * 128:(sb + 1) * 128],
                                    vp_tiles[tb][:, h * 65:(h + 1) * 65],
                                    start=(tb == 0), stop=(tb == NTB - 1))
                        r1 = sp.tile([128, 1], f32, tag="r1", name="r1")
                        nc.vector.reciprocal(r1[:], bank[:, 64:65])
                        nc.vector.tensor_scalar(
                            ctxn[sb][:, h, :], slot[:, 0:HD], r1[:],
                            None, ALU.mult)
                    # [128s, (2h,64d)] -> [(2h,64d), 128s] on the DMA xbar
                    nc.sync.dma_start_transpose(
                        ctxTs[:, sb * 128:(sb + 1) * 128], ctxn[sb][:])
                for sb in range(4):
                    col = qc * QC + sb * 128
                    nc.tensor.matmul(mix[:, :], ctxTs[:, sb * 128:(sb + 1) * 128],
                                     wo_sb[:], start=True, stop=True)
                    ob = obp.tile([128, HID], f32, tag="ob", name="ob")
                    nc.scalar.copy(ob[:], mix[:, :])
                    nc.sync.dma_start(out[col:col + 128, :], ob[:])

            # ---------------- phase A: projections + qc0/qc1 QK+exp --------
            for t8 in range(NQC):
                cs = slice(t8 * QC, (t8 + 1) * QC)
                xcs = []
                for i in range(4):
                    xc = wp.tile([128, QC], bf16, tag="xc", bufs=8)
                    nc.sync.dma_start(xc[:], xT[i * 128:(i + 1) * 128, cs])
                    xcs.append(xc)
                for w_t, bias_sb, dst in ((wk_t, bk_sb, kt), (wq_t, bq_sb, qt)):
                    for i in range(4):
                        nc.tensor.matmul(
                            mix[:, :], w_t[:, i, :], xcs[i][:],
                            start=(i == 0), stop=(i == 3))
                    nc.vector.tensor_scalar_add(dst[:, cs], mix[:, :], bias_sb[:])
                for tl in range(4):
                    vps = vpb[:, 0:130]
                    for i in range(4):
                        nc.tensor.matmul(
                            vps, xcs[i][:, tl * 128:(tl + 1) * 128], wv_t[:, i, :],
                            start=(i == 0), stop=(i == 3))
                    vp = vpp.tile([128, 130], bf16, tag="vp")
                    nc.vector.tensor_tensor(vp[:], vps, bvb[:], ALU.add)
                    vp_tiles.append(vp)
                # qc0 scores/exp ride along as soon as K/Q/V chunks exist
                for tb2 in (2 * t8, 2 * t8 + 1):
                    for h in range(2):
                        emit_qk_exp(0, h, tb2)
                # late projection: qc1's first score/exp groups keep the
                # exp engines fed
                if t8 >= NQC - Q1_EARLY:
                    tb2e = t8 - (NQC - Q1_EARLY)
                    for h in range(2):
                        emit_qk_exp(1, h, tb2e)
            emit_pv_tail(0)

            # ---------------- phase B: attention + output projection -------
            for qc in range(1, NQC):
                tb2_start = Q1_EARLY if qc == 1 else 0
                for tb2 in range(tb2_start, NTB // 2):
                    for h in range(2):
                        emit_qk_exp(qc, h, tb2)
                emit_pv_tail(qc)

    nc.compile()
    return nc


def _get_program():
    global _PROGRAM
    if _PROGRAM is None:
        _PROGRAM = _build_program()
    return _PROGRAM


def kernel(**inputs):
    _ensure_imports()
    import ml_dtypes
    from concourse import bass_utils

    bf = ml_dtypes.bfloat16
    x = np.ascontiguousarray(np.asarray(inputs["x"], dtype=np.float32))
    Wq = np.asarray(inputs["Wq"], dtype=np.float32)
    Wk = np.asarray(inputs["Wk"], dtype=np.float32)
    Wv = np.asarray(inputs["Wv"], dtype=np.float32)
    Wo = np.asarray(inputs["Wo"], dtype=np.float32)
    bq = np.asarray(inputs["bq"], dtype=np.float32)
    bk = np.asarray(inputs["bk"], dtype=np.float32)
    bv = np.asarray(inputs["bv"], dtype=np.float32)
    bo = np.asarray(inputs["bo"], dtype=np.float32)

    nc = _get_program()

    wqT_full = np.ascontiguousarray(Wq.T)
    wkT_full = np.ascontiguousarray(Wk.T)
    wvT_full = np.ascontiguousarray(Wv.T)
    woT_full = np.ascontiguousarray(Wo.T)

    in_maps = []
    for c in range(NCORES):
        b, g = divmod(c, 4)
        fs = slice(g * FSL, (g + 1) * FSL)
        wv_aug = np.zeros((HID, 130), np.float32)
        wv_aug[:, 0:64] = wvT_full[:, g * FSL:g * FSL + 64]
        wv_aug[:, 65:129] = wvT_full[:, g * FSL + 64:(g + 1) * FSL]
        bv_aug = np.zeros((1, 130), np.float32)
        bv_aug[0, 0:64] = bv[g * FSL:g * FSL + 64]
        bv_aug[0, 64] = 1.0
        bv_aug[0, 65:129] = bv[g * FSL + 64:(g + 1) * FSL]
        bv_aug[0, 129] = 1.0
        in_maps.append({
            "xT": np.ascontiguousarray(x[b].T.astype(bf)),
            "wqT": np.ascontiguousarray(wqT_full[:, fs].astype(bf)),
            "wkT": np.ascontiguousarray(wkT_full[:, fs].astype(bf)),
            "wvT": np.ascontiguousarray(wv_aug.astype(bf)),
            "woT": np.ascontiguousarray(woT_full[fs, :].astype(bf)),
            "bq": np.ascontiguousarray(bq[fs].reshape(FSL, 1)),
            "bk": np.ascontiguousarray(bk[fs].reshape(FSL, 1)),
            "bvr": bv_aug,
        })

    res = bass_utils.run_bass_kernel_spmd(nc, in_maps,
                                          core_ids=list(range(NCORES)))
    outs = [np.asarray(r["out"], dtype=np.float32) for r in res.results]

    full = np.empty((B, S, HID), dtype=np.float32)
    for b in range(B):
        full[b] = outs[4 * b] + outs[4 * b + 1] + outs[4 * b + 2] + outs[4 * b + 3]
        full[b] += bo
    return full


# revision 9
# speedup vs baseline: 1.1054x; 1.0073x over previous
"""Multi-head attention (B=2, S=4096, HIDDEN=512, HEADS=8) on 8 TRN2 NeuronCores.

Sharding: 8 cores = 2 batches x 4 head-groups (2 heads each).
Core c handles batch b = c//4 and heads {2g, 2g+1} where g = c%4
(projection feature slice [g*128, (g+1)*128)).

Per-core kernel, all-bf16 matmul datapath (host pre-converts x and the
weights to bf16; PSUM accumulation stays fp32):
  - K^T/Q^T projections from x^T chunks (lhsT = W chunks, N=512)
  - V projected directly into natural [t, d] layout (lhsT = x^T chunk
    slices), with zero-padded W columns + a broadcast bias row so the
    per-head ones column (softmax denominator) appears for free
  - scores S^T[t, s] = K^T-block^T Q^T chunk in PSUM (fp32, exact)
  - softmax exp split across two engines:
      ACT:  exact exp activation (fp32 -> bf16)
      DVE:  one-pass Schraudolph exp2: bf16_bitcast(int16(
        x*(16*log2e) + B)) -- the f32->int16 convert rounds to nearest,
        the int16 bit pattern IS the bf16 exponential approximation
  - PV in swapped orientation: ctx[s, d+1] accumulates pt-block^T @ vp,
    N=65 per matmul (full 128-wide stationary dim) -- half the PE rows
    of the [d, s] orientation.  PSUM accumulation groups are 2KB-bank
    granular in the sim (start=True wipes the whole bank row), so each
    (query-block, head) group runs alone on its bank, all groups of a
    bank at the same columns so the hazard tracker serializes them.
  - normalize: l sits in ctx column 64; per-partition reciprocal +
    tensor_scalar multiply (queries live on partitions here)
  - ctx transposed back to [d, s] by the DMA crossbar (2-byte
    transpose, no PE/DVE cost)
  - output projection per 128-query block -> partial [S, 512]
Host sums the 4 partials per batch and adds bo.
"""

import sys

import numpy as np

B, S, HID, HEADS, HD = 2, 4096, 512, 8, 64
FSL = 128          # features per core = 2 heads * 64
NCORES = 8
QC = 512           # query-chunk width
NTB = S // 128     # 32 key blocks
NQC = S // QC      # 8 query chunks
Q1_EARLY = 4       # qc=1 tb2-pairs whose QK/exp ride the projection phase

LOG2E = 1.4426950408889634
EXP_S = 16.0 * LOG2E          # 128*log2e * scale(1/8)
EXP_B = 16256.0 - 7.5         # zero-mean-tuned Schraudolph constant

# exp engine assignment pattern, indexed by a global tile counter.
# A = ACT exact exp, D = DVE schraudolph (GPSIMD cannot read PSUM).
EXP_PATTERN = ("A", "A", "D", "A", "D", "A", "A", "D", "A", "D", "A", "D", "A")

_PROGRAM = None


def _ensure_imports():
    try:
        import concourse  # noqa: F401
    except ImportError:
        sys.path.insert(0, "/opt/trn_rl_repo")


def _build_program():
    _ensure_imports()
    import concourse.bacc as bacc
    import concourse.mybir as mybir
    import concourse.tile as tile

    f32 = mybir.dt.float32
    bf16 = mybir.dt.bfloat16
    i16 = mybir.dt.int16
    AF = mybir.ActivationFunctionType
    ALU = mybir.AluOpType

    nc = bacc.Bacc(
        "TRN2",
        target_bir_lowering=False,
        debug=False,
        enable_asserts=False,
        num_devices=NCORES,
    )

    xT = nc.dram_tensor("xT", [HID, S], bf16, kind="ExternalInput").ap()
    wqT = nc.dram_tensor("wqT", [HID, FSL], bf16, kind="ExternalInput").ap()
    wkT = nc.dram_tensor("wkT", [HID, FSL], bf16, kind="ExternalInput").ap()
    wvT = nc.dram_tensor("wvT", [HID, 130], bf16, kind="ExternalInput").ap()
    woT = nc.dram_tensor("woT", [FSL, HID], bf16, kind="ExternalInput").ap()
    bq = nc.dram_tensor("bq", [FSL, 1], f32, kind="ExternalInput").ap()
    bk = nc.dram_tensor("bk", [FSL, 1], f32, kind="ExternalInput").ap()
    bvr = nc.dram_tensor("bvr", [1, 130], f32, kind="ExternalInput").ap()
    out = nc.dram_tensor("out", [S, HID], f32, kind="ExternalOutput").ap()

    exp_idx = [0]
    pt_map = {}

    with tile.TileContext(nc) as tc:
        with (
            tc.tile_pool(name="persist", bufs=1) as pp,
            tc.tile_pool(name="vp_pool", bufs=NTB) as vpp,
            tc.tile_pool(name="pspersist", bufs=1, space="PSUM") as psp,
            tc.tile_pool(name="stp", bufs=2, space="PSUM") as stp,
            tc.tile_pool(name="pt_pool", bufs=44) as ptp,
            tc.tile_pool(name="w_pool", bufs=1) as wp,
            tc.tile_pool(name="small", bufs=4) as sp,
            tc.tile_pool(name="out_pool", bufs=4) as obp,
        ):
            kt = pp.tile([FSL, S], bf16, tag="kt")
            qt = pp.tile([FSL, S], bf16, tag="qt")
            wo_sb = pp.tile([FSL, HID], bf16, tag="wo_sb")
            bq_sb = pp.tile([FSL, 1], f32, tag="bq_sb")
            bk_sb = pp.tile([FSL, 1], f32, tag="bk_sb")
            bvr_sb = pp.tile([1, 130], f32, tag="bvr_sb")
            bvb = pp.tile([128, 130], f32, tag="bvb")

            # persistent PSUM banks; one live accumulation group per bank
            # (sim group state is bank-granular):
            #  ctxA/ctxB: PV slot groups, always at cols 0:65
            #  mix: K/Q projection psum (phase A), po output proj (tails)
            #  vpb: V projection psum (phase A)
            ctxA = psp.tile([128, 512], f32, tag="ctxA")
            ctxB = psp.tile([128, 512], f32, tag="ctxB")
            mix = psp.tile([128, 512], f32, tag="mix")
            vpb = psp.tile([128, 512], f32, tag="vpb")
            # vpb doubles as a third ctx bank once projections are done
            # (the PV slot range overlaps the V-psum range, so the hazard
            # tracker orders the phases)
            ctx_banks = (ctxA, ctxB, vpb)

            # small DMAs ride the SWDGE ring
            nc.gpsimd.dma_start(bq_sb[:], bq[:])
            nc.gpsimd.dma_start(bk_sb[:], bk[:])
            nc.gpsimd.dma_start(bvr_sb[:], bvr[:])
            nc.gpsimd.partition_broadcast(bvb[:], bvr_sb[0:1, :])

            # weights first on the sync ring: tiny, needed before projections
            wk_t = wp.tile([128, 4, FSL], bf16, tag="wk_t")
            nc.sync.dma_start(wk_t[:], wkT.rearrange("(i p) f -> p i f", p=128))
            wq_t = wp.tile([128, 4, FSL], bf16, tag="wq_t")
            nc.sync.dma_start(wq_t[:], wqT.rearrange("(i p) f -> p i f", p=128))
            wv_t = wp.tile([128, 4, 130], bf16, tag="wv_t")
            nc.sync.dma_start(wv_t[:], wvT.rearrange("(i p) f -> p i f", p=128))
            nc.sync.dma_start(wo_sb[:], woT[:])

            vp_tiles = []

            def emit_qk_exp(qc, h, tb2):
                st = stp.tile([128, 2, QC], f32, tag="st", name="st")
                for j in range(2):
                    tb = tb2 * 2 + j
                    nc.tensor.matmul(
                        st[:, j, :],
                        kt[h * HD:(h + 1) * HD, tb * 128:(tb + 1) * 128],
                        qt[h * HD:(h + 1) * HD, qc * QC:(qc + 1) * QC],
                        start=True, stop=True)
                pt = ptp.tile([128, 2, QC], bf16, tag="pt", name="pt")
                eng = EXP_PATTERN[exp_idx[0] % len(EXP_PATTERN)]
                exp_idx[0] += 1
                if eng == "A":
                    nc.scalar.activation(pt[:], st[:], AF.Exp,
                                         scale=float(HD) ** -0.5)
                else:
                    nc.vector.tensor_scalar(pt[:].bitcast(i16), st[:],
                                            EXP_S, EXP_B, ALU.mult, ALU.add)
                pt_map[(qc, h, tb2)] = pt

            def emit_pv_tail(qc, filler=()):
                # filler: (qc', h, tb2) QK/exp emissions interleaved between
                # PV groups so PE/ACT/DVE stay fed while the per-group
                # recip/normalize round-trips drain
                filler = list(filler)
                nfill = len(filler)
                fi = 0
                ctxn = [sp.tile([128, 2, HD], bf16, tag=f"cn{sb}",
                                name=f"cn{sb}") for sb in range(4)]
                ctxTs = sp.tile([128, 512], bf16, tag="ctxTs", name="ctxTs")
                for g in range(8):
                    sb, h = g // 2, g % 2
                    bank = ctx_banks[g % 3]
                    slot = bank[:, 0:65]
                    for tb2 in range(NTB // 2):
                        pt = pt_map[(qc, h, tb2)]
                        for j in range(2):
                            tb = tb2 * 2 + j
                            nc.tensor.matmul(
                                slot,
                                pt[:, j, sb * 128:(sb + 1) * 128],
                                vp_tiles[tb][:, h * 65:(h + 1) * 65],
                                start=(tb == 0), stop=(tb == NTB - 1))
                    r1 = sp.tile([128, 1], f32, tag="r1", name="r1")
                    nc.vector.reciprocal(r1[:], bank[:, 64:65])
                    nc.vector.tensor_scalar(
                        ctxn[sb][:, h, :], slot[:, 0:HD], r1[:],
                        None, ALU.mult)
                    if h == 1:
                        # [128s, (2h,64d)] -> [(2h,64d), 128s] on the DMA xbar
                        nc.sync.dma_start_transpose(
                            ctxTs[:, sb * 128:(sb + 1) * 128], ctxn[sb][:])
                        col = qc * QC + sb * 128
                        nc.tensor.matmul(
                            mix[:, :], ctxTs[:, sb * 128:(sb + 1) * 128],
                            wo_sb[:], start=True, stop=True)
                        ob = obp.tile([128, HID], f32, tag="ob", name="ob")
                        nc.scalar.copy(ob[:], mix[:, :])
                        nc.sync.dma_start(out[col:col + 128, :], ob[:])
                    want = nfill * (g + 1) // 8
                    while fi < want:
                        emit_qk_exp(*filler[fi])
                        fi += 1

            # ---------------- phase A: projections + qc0/qc1 QK+exp --------
            for t8 in range(NQC):
                cs = slice(t8 * QC, (t8 + 1) * QC)
                xcs = []
                for i in range(4):
                    xc = wp.tile([128, QC], bf16, tag="xc", bufs=8)
                    nc.sync.dma_start(xc[:], xT[i * 128:(i + 1) * 128, cs])
                    xcs.append(xc)
                for w_t, bias_sb, dst in ((wk_t, bk_sb, kt), (wq_t, bq_sb, qt)):
                    for i in range(4):
                        nc.tensor.matmul(
                            mix[:, :], w_t[:, i, :], xcs[i][:],
                            start=(i == 0), stop=(i == 3))
                    nc.vector.tensor_scalar_add(dst[:, cs], mix[:, :], bias_sb[:])
                for tl in range(4):
                    vps = vpb[:, 0:130]
                    for i in range(4):
                        nc.tensor.matmul(
                            vps, xcs[i][:, tl * 128:(tl + 1) * 128], wv_t[:, i, :],
                            start=(i == 0), stop=(i == 3))
                    vp = vpp.tile([128, 130], bf16, tag="vp")
                    nc.vector.tensor_tensor(vp[:], vps, bvb[:], ALU.add)
                    vp_tiles.append(vp)
                # qc0 scores/exp ride along as soon as K/Q/V chunks exist
                for tb2 in (2 * t8, 2 * t8 + 1):
                    for h in range(2):
                        emit_qk_exp(0, h, tb2)
                # late projection: qc1's first score/exp groups keep the
                # exp engines fed
                if t8 >= NQC - Q1_EARLY:
                    tb2e = t8 - (NQC - Q1_EARLY)
                    for h in range(2):
                        emit_qk_exp(1, h, tb2e)
            # ---------------- phase B: attention + output projection -------
            for qc in range(NQC):
                filler = []
                if qc + 1 < NQC:
                    ts = Q1_EARLY if qc + 1 == 1 else 0
                    filler = [(qc + 1, h, t)
                              for t in range(ts, NTB // 2) for h in range(2)]
                emit_pv_tail(qc, filler)

    nc.compile()
    return nc


def _get_program():
    global _PROGRAM
    if _PROGRAM is None:
        _PROGRAM = _build_program()
    return _PROGRAM


def kernel(**inputs):
    _ensure_imports()
    import ml_dtypes
    from concourse import bass_utils

    bf = ml_dtypes.bfloat16
    x = np.ascontiguousarray(np.asarray(inputs["x"], dtype=np.float32))
    Wq = np.asarray(inputs["Wq"], dtype=np.float32)
    Wk = np.asarray(inputs["Wk"], dtype=np.float32)
    Wv = np.asarray(inputs["Wv"], dtype=np.float32)
    Wo = np.asarray(inputs["Wo"], dtype=np.float32)
    bq = np.asarray(inputs["bq"], dtype=np.float32)
    bk = np.asarray(inputs["bk"], dtype=np.float32)
    bv = np.asarray(inputs["bv"], dtype=np.float32)
    bo = np.asarray(inputs["bo"], dtype=np.float32)

    nc = _get_program()

    wqT_full = np.ascontiguousarray(Wq.T)
    wkT_full = np.ascontiguousarray(Wk.T)
    wvT_full = np.ascontiguousarray(Wv.T)
    woT_full = np.ascontiguousarray(Wo.T)

    in_maps = []
    for c in range(NCORES):
        b, g = divmod(c, 4)
        fs = slice(g * FSL, (g + 1) * FSL)
        wv_aug = np.zeros((HID, 130), np.float32)
        wv_aug[:, 0:64] = wvT_full[:, g * FSL:g * FSL + 64]
        wv_aug[:, 65:129] = wvT_full[:, g * FSL + 64:(g + 1) * FSL]
        bv_aug = np.zeros((1, 130), np.float32)
        bv_aug[0, 0:64] = bv[g * FSL:g * FSL + 64]
        bv_aug[0, 64] = 1.0
        bv_aug[0, 65:129] = bv[g * FSL + 64:(g + 1) * FSL]
        bv_aug[0, 129] = 1.0
        in_maps.append({
            "xT": np.ascontiguousarray(x[b].T.astype(bf)),
            "wqT": np.ascontiguousarray(wqT_full[:, fs].astype(bf)),
            "wkT": np.ascontiguousarray(wkT_full[:, fs].astype(bf)),
            "wvT": np.ascontiguousarray(wv_aug.astype(bf)),
            "woT": np.ascontiguousarray(woT_full[fs, :].astype(bf)),
            "bq": np.ascontiguousarray(bq[fs].reshape(FSL, 1)),
            "bk": np.ascontiguousarray(bk[fs].reshape(FSL, 1)),
            "bvr": bv_aug,
        })

    res = bass_utils.run_bass_kernel_spmd(nc, in_maps,
                                          core_ids=list(range(NCORES)))
    outs = [np.asarray(r["out"], dtype=np.float32) for r in res.results]

    full = np.empty((B, S, HID), dtype=np.float32)
    for b in range(B):
        full[b] = outs[4 * b] + outs[4 * b + 1] + outs[4 * b + 2] + outs[4 * b + 3]
        full[b] += bo
    return full


# revision 17
# speedup vs baseline: 1.1950x; 1.0811x over previous
"""Multi-head attention (B=2, S=4096, HIDDEN=512, HEADS=8) on 8 TRN2 NeuronCores.

Sharding: 8 cores = 2 batches x 4 head-groups (2 heads each).
Core c handles batch b = c//4 and heads {2g, 2g+1} where g = c%4
(projection feature slice [g*128, (g+1)*128)).

Per-core kernel, all-bf16 matmul datapath (host pre-converts x and the
weights to bf16; PSUM accumulation stays fp32):
  - K^T/Q^T projections from x^T chunks (lhsT = W chunks, N=512)
  - V projected directly into natural [t, d] layout (lhsT = x^T chunk
    slices), with zero-padded W columns + a broadcast bias row so the
    per-head ones column (softmax denominator) appears for free
  - scores S^T[t, s] = K^T-block^T Q^T chunk in PSUM (fp32, exact)
  - softmax exp split across two engines:
      ACT:  exact exp activation (fp32 -> bf16)
      DVE:  one-pass Schraudolph exp2: bf16_bitcast(int16(
        x*(16*log2e) + B)) -- the f32->int16 convert rounds to nearest,
        the int16 bit pattern IS the bf16 exponential approximation
  - PV in swapped orientation: ctx[s, d+1] accumulates pt-block^T @ vp,
    N=65 per matmul (full 128-wide stationary dim) -- half the PE rows
    of the [d, s] orientation.  PSUM accumulation groups are 2KB-bank
    granular in the sim (start=True wipes the whole bank row), so each
    (query-block, head) group runs alone on its bank, all groups of a
    bank at the same columns so the hazard tracker serializes them.
  - normalize: l sits in ctx column 64; per-partition reciprocal +
    tensor_scalar multiply (queries live on partitions here)
  - ctx transposed back to [d, s] by the DMA crossbar (2-byte
    transpose, no PE/DVE cost)
  - output projection per 128-query block -> partial [S, 512]
Host sums the 4 partials per batch and adds bo.
"""

import sys

import numpy as np

B, S, HID, HEADS, HD = 2, 4096, 512, 8, 64
FSL = 128          # features per core = 2 heads * 64
NCORES = 8
QC = 512           # query-chunk width
NTB = S // 128     # 32 key blocks
NQC = S // QC      # 8 query chunks
Q1_EARLY = 4       # qc=1 tb2-pairs whose QK/exp ride the projection phase

LOG2E = 1.4426950408889634
EXP_S = 16.0 * LOG2E          # 128*log2e * scale(1/8)
EXP_B = 16256.0 - 7.5         # zero-mean-tuned Schraudolph constant

# exp engine assignment pattern, indexed by a global tile counter.
# A = ACT exact exp, D = DVE schraudolph (GPSIMD cannot read PSUM).
EXP_PATTERN = ("A", "A", "D", "A", "D", "A", "A", "D", "A", "D", "A", "D", "A")

_PROGRAM = None


def _ensure_imports():
    try:
        import concourse  # noqa: F401
    except ImportError:
        sys.path.insert(0, "/opt/trn_rl_repo")


def _build_program():
    _ensure_imports()
    import concourse.bacc as bacc
    import concourse.mybir as mybir
    import concourse.tile as tile

    f32 = mybir.dt.float32
    bf16 = mybir.dt.bfloat16
    i16 = mybir.dt.int16
    AF = mybir.ActivationFunctionType
    ALU = mybir.AluOpType

    nc = bacc.Bacc(
        "TRN2",
        target_bir_lowering=False,
        debug=False,
        enable_asserts=False,
        num_devices=NCORES,
    )

    xT = nc.dram_tensor("xT", [HID, S], bf16, kind="ExternalInput").ap()
    wqT = nc.dram_tensor("wqT", [HID, FSL], bf16, kind="ExternalInput").ap()
    wkT = nc.dram_tensor("wkT", [HID, FSL], bf16, kind="ExternalInput").ap()
    wvT = nc.dram_tensor("wvT", [HID, 130], bf16, kind="ExternalInput").ap()
    woT = nc.dram_tensor("woT", [FSL, HID], bf16, kind="ExternalInput").ap()
    bq = nc.dram_tensor("bq", [FSL, 1], f32, kind="ExternalInput").ap()
    bk = nc.dram_tensor("bk", [FSL, 1], f32, kind="ExternalInput").ap()
    bvr = nc.dram_tensor("bvr", [1, 130], f32, kind="ExternalInput").ap()
    out = nc.dram_tensor("out", [S, HID], f32, kind="ExternalOutput").ap()

    exp_idx = [0]
    pt_map = {}

    with tile.TileContext(nc) as tc:
        with (
            tc.tile_pool(name="persist", bufs=1) as pp,
            tc.tile_pool(name="vp_pool", bufs=NTB) as vpp,
            tc.tile_pool(name="pspersist", bufs=1, space="PSUM") as psp,
            tc.tile_pool(name="stp", bufs=2, space="PSUM") as stp,
            tc.tile_pool(name="pt_pool", bufs=44) as ptp,
            tc.tile_pool(name="w_pool", bufs=1) as wp,
            tc.tile_pool(name="small", bufs=4) as sp,
            tc.tile_pool(name="out_pool", bufs=4) as obp,
        ):
            kt = pp.tile([FSL, S], bf16, tag="kt")
            qt = pp.tile([FSL, S], bf16, tag="qt")
            wo_sb = pp.tile([FSL, HID], bf16, tag="wo_sb")
            bq_sb = pp.tile([FSL, 1], f32, tag="bq_sb")
            bk_sb = pp.tile([FSL, 1], f32, tag="bk_sb")
            bvr_sb = pp.tile([1, 130], f32, tag="bvr_sb")
            bvb = pp.tile([128, 130], f32, tag="bvb")

            # persistent PSUM banks; one live accumulation group per bank
            # (sim group state is bank-granular):
            #  ctxA/ctxB: PV slot groups, always at cols 0:65
            #  mix: K/Q projection psum (phase A), po output proj (tails)
            #  vpb: V projection psum (phase A)
            ctxA = psp.tile([128, 512], f32, tag="ctxA")
            ctxB = psp.tile([128, 512], f32, tag="ctxB")
            mix = psp.tile([128, 512], f32, tag="mix")
            # ctxB doubles as the V-projection psum during phase A (the PV
            # slot range overlaps the V-psum range, so the hazard tracker
            # orders the phases)
            ctx_banks = (ctxA, ctxB)

            # small DMAs ride the SWDGE ring
            nc.gpsimd.dma_start(bq_sb[:], bq[:])
            nc.gpsimd.dma_start(bk_sb[:], bk[:])
            nc.gpsimd.dma_start(bvr_sb[:], bvr[:])
            nc.gpsimd.partition_broadcast(bvb[:], bvr_sb[0:1, :])

            # weights first on the sync ring: tiny, needed before projections
            wk_t = wp.tile([128, 4, FSL], bf16, tag="wk_t")
            nc.sync.dma_start(wk_t[:], wkT.rearrange("(i p) f -> p i f", p=128))
            wq_t = wp.tile([128, 4, FSL], bf16, tag="wq_t")
            nc.sync.dma_start(wq_t[:], wqT.rearrange("(i p) f -> p i f", p=128))
            wv_t = wp.tile([128, 4, 130], bf16, tag="wv_t")
            nc.sync.dma_start(wv_t[:], wvT.rearrange("(i p) f -> p i f", p=128))
            nc.sync.dma_start(wo_sb[:], woT[:])

            vp_tiles = []

            def emit_qk_exp(qc, h, tbs):
                # one score tile covering t-blocks `tbs` (len 1 or 2); the
                # 5-bank [pair, pair, single] rotation gives pipeline depth
                # ~3 exp units so QK never waits on the exp round-trip
                if len(tbs) == 2:
                    st = stp.tile([128, 2, QC], f32, tag="stA", bufs=2,
                                  name="st")
                    pt = ptp.tile([128, 2, QC], bf16, tag="ptA", bufs=56,
                                  name="pt")
                    st_js = [st[:, j, :] for j in range(2)]
                    pt_js = [pt[:, j, :] for j in range(2)]
                else:
                    st = stp.tile([128, QC], f32, tag="stB", bufs=1,
                                  name="st")
                    pt = ptp.tile([128, QC], bf16, tag="ptB", bufs=26,
                                  name="pt")
                    st_js = [st[:]]
                    pt_js = [pt[:]]
                for j, tb in enumerate(tbs):
                    nc.tensor.matmul(
                        st_js[j],
                        kt[h * HD:(h + 1) * HD, tb * 128:(tb + 1) * 128],
                        qt[h * HD:(h + 1) * HD, qc * QC:(qc + 1) * QC],
                        start=True, stop=True)
                eng = EXP_PATTERN[exp_idx[0] % len(EXP_PATTERN)]
                exp_idx[0] += 1
                if eng == "A":
                    nc.scalar.activation(pt[:], st[:], AF.Exp,
                                         scale=float(HD) ** -0.5)
                else:
                    nc.vector.tensor_scalar(pt[:].bitcast(i16), st[:],
                                            EXP_S, EXP_B, ALU.mult, ALU.add)
                for j, tb in enumerate(tbs):
                    pt_map[(qc, h, tb)] = pt_js[j]

            def emit_pv_tail(qc, filler=()):
                # filler: (qc', h, tb2) QK/exp emissions interleaved between
                # PV groups so PE/ACT/DVE stay fed while the per-group
                # recip/normalize round-trips drain
                filler = list(filler)
                nfill = len(filler)
                fi = 0
                ctxn = [sp.tile([128, 2, HD], bf16, tag=f"cn{sb}",
                                name=f"cn{sb}") for sb in range(4)]
                ctxTs = sp.tile([128, 512], bf16, tag="ctxTs", name="ctxTs")
                for g in range(8):
                    sb, h = g // 2, g % 2
                    bank = ctx_banks[g % 2]
                    slot = bank[:, 0:65]
                    for tb in range(NTB):
                        pt = pt_map[(qc, h, tb)]
                        nc.tensor.matmul(
                            slot,
                            pt[:, sb * 128:(sb + 1) * 128],
                            vp_tiles[tb][:, h * 65:(h + 1) * 65],
                            start=(tb == 0), stop=(tb == NTB - 1))
                    r1 = sp.tile([128, 1], f32, tag="r1", name="r1")
                    nc.vector.reciprocal(r1[:], bank[:, 64:65])
                    nc.vector.tensor_scalar(
                        ctxn[sb][:, h, :], slot[:, 0:HD], r1[:],
                        None, ALU.mult)
                    if h == 1:
                        # [128s, (2h,64d)] -> [(2h,64d), 128s] on the DMA xbar
                        nc.sync.dma_start_transpose(
                            ctxTs[:, sb * 128:(sb + 1) * 128], ctxn[sb][:])
                        col = qc * QC + sb * 128
                        nc.tensor.matmul(
                            mix[:, :], ctxTs[:, sb * 128:(sb + 1) * 128],
                            wo_sb[:], start=True, stop=True)
                        ob = obp.tile([128, HID], f32, tag="ob", name="ob")
                        nc.scalar.copy(ob[:], mix[:, :])
                        nc.sync.dma_start(out[col:col + 128, :], ob[:])
                    want = nfill * (g + 1) // 8
                    while fi < want:
                        emit_qk_exp(*filler[fi])
                        fi += 1

            # ---------------- phase A: projections + qc0/qc1 QK+exp --------
            for t8 in range(NQC):
                cs = slice(t8 * QC, (t8 + 1) * QC)
                xcs = []
                for i in range(4):
                    xc = wp.tile([128, QC], bf16, tag="xc", bufs=8)
                    nc.sync.dma_start(xc[:], xT[i * 128:(i + 1) * 128, cs])
                    xcs.append(xc)
                for w_t, bias_sb, dst in ((wk_t, bk_sb, kt), (wq_t, bq_sb, qt)):
                    for i in range(4):
                        nc.tensor.matmul(
                            mix[:, :], w_t[:, i, :], xcs[i][:],
                            start=(i == 0), stop=(i == 3))
                    nc.vector.tensor_scalar_add(dst[:, cs], mix[:, :], bias_sb[:])
                for tl in range(4):
                    vps = ctxB[:, 0:130]
                    for i in range(4):
                        nc.tensor.matmul(
                            vps, xcs[i][:, tl * 128:(tl + 1) * 128], wv_t[:, i, :],
                            start=(i == 0), stop=(i == 3))
                    vp = vpp.tile([128, 130], bf16, tag="vp")
                    nc.vector.tensor_tensor(vp[:], vps, bvb[:], ALU.add)
                    vp_tiles.append(vp)
                # qc0 scores/exp ride along as soon as K/Q/V chunks exist
                for tb2 in (2 * t8, 2 * t8 + 1):
                    for h in range(2):
                        emit_qk_exp(0, h, (2 * tb2, 2 * tb2 + 1))
                # late projection: qc1's first score/exp groups keep the
                # exp engines fed
                if t8 >= NQC - Q1_EARLY:
                    tb2e = t8 - (NQC - Q1_EARLY)
                    for h in range(2):
                        emit_qk_exp(1, h, (2 * tb2e, 2 * tb2e + 1))
            # ---------------- phase B: attention + output projection -------
            def chunked(tb0):
                # [pair, pair, single] rotation from t-block tb0 to NTB
                cyc = [2, 2, 1]
                out_, i, tb = [], 0, tb0
                while tb < NTB:
                    n = min(cyc[i % 3], NTB - tb)
                    out_.append(tuple(range(tb, tb + n)))
                    tb += n
                    i += 1
                return out_

            for qc in range(NQC):
                filler = []
                if qc + 1 < NQC:
                    ts = 2 * Q1_EARLY if qc + 1 == 1 else 0
                    filler = [(qc + 1, h, tbs)
                              for tbs in chunked(ts) for h in range(2)]
                emit_pv_tail(qc, filler)

    nc.compile()
    return nc


def _get_program():
    global _PROGRAM
    if _PROGRAM is None:
        _PROGRAM = _build_program()
    return _PROGRAM


def kernel(**inputs):
    _ensure_imports()
    import ml_dtypes
    from concourse import bass_utils

    bf = ml_dtypes.bfloat16
    x = np.ascontiguousarray(np.asarray(inputs["x"], dtype=np.float32))
    Wq = np.asarray(inputs["Wq"], dtype=np.float32)
    Wk = np.asarray(inputs["Wk"], dtype=np.float32)
    Wv = np.asarray(inputs["Wv"], dtype=np.float32)
    Wo = np.asarray(inputs["Wo"], dtype=np.float32)
    bq = np.asarray(inputs["bq"], dtype=np.float32)
    bk = np.asarray(inputs["bk"], dtype=np.float32)
    bv = np.asarray(inputs["bv"], dtype=np.float32)
    bo = np.asarray(inputs["bo"], dtype=np.float32)

    nc = _get_program()

    wqT_full = np.ascontiguousarray(Wq.T)
    wkT_full = np.ascontiguousarray(Wk.T)
    wvT_full = np.ascontiguousarray(Wv.T)
    woT_full = np.ascontiguousarray(Wo.T)

    in_maps = []
    for c in range(NCORES):
        b, g = divmod(c, 4)
        fs = slice(g * FSL, (g + 1) * FSL)
        wv_aug = np.zeros((HID, 130), np.float32)
        wv_aug[:, 0:64] = wvT_full[:, g * FSL:g * FSL + 64]
        wv_aug[:, 65:129] = wvT_full[:, g * FSL + 64:(g + 1) * FSL]
        bv_aug = np.zeros((1, 130), np.float32)
        bv_aug[0, 0:64] = bv[g * FSL:g * FSL + 64]
        bv_aug[0, 64] = 1.0
        bv_aug[0, 65:129] = bv[g * FSL + 64:(g + 1) * FSL]
        bv_aug[0, 129] = 1.0
        in_maps.append({
            "xT": np.ascontiguousarray(x[b].T.astype(bf)),
            "wqT": np.ascontiguousarray(wqT_full[:, fs].astype(bf)),
            "wkT": np.ascontiguousarray(wkT_full[:, fs].astype(bf)),
            "wvT": np.ascontiguousarray(wv_aug.astype(bf)),
            "woT": np.ascontiguousarray(woT_full[fs, :].astype(bf)),
            "bq": np.ascontiguousarray(bq[fs].reshape(FSL, 1)),
            "bk": np.ascontiguousarray(bk[fs].reshape(FSL, 1)),
            "bvr": bv_aug,
        })

    res = bass_utils.run_bass_kernel_spmd(nc, in_maps,
                                          core_ids=list(range(NCORES)))
    outs = [np.asarray(r["out"], dtype=np.float32) for r in res.results]

    full = np.empty((B, S, HID), dtype=np.float32)
    for b in range(B):
        full[b] = outs[4 * b] + outs[4 * b + 1] + outs[4 * b + 2] + outs[4 * b + 3]
        full[b] += bo
    return full
